# revision 29
# baseline (speedup 1.0000x reference)
"""Transformer layer (attention with materialized probs + FFN) on 8 Trainium2
NeuronCores, data-parallel over the batch (B=8, one batch element per core).

Layout strategy: everything on-device is kept transposed ([feature, seq]) so
every matmul contracts over the SBUF partition dimension with weights in
natural (host-pre-transposed/packed) layout. Attention probs are produced as
probsT[h, sk, sq] and transposed back on the host during unsharding; x2 is
produced as x2T[d, s]. Matmuls run in float32r (TF32-like, ~1e-4 rel err,
full bf16-rate on the PE). Softmax skips the max-subtraction (scores are
O(1) by construction: q is pre-scaled by 1/sqrt(HD)); denominators come for
free from an all-ones 65th column appended to V in the probs@V matmul.
LayerNorm is applied as x*A + B with rank-1/rank-2 broadcast matmuls
(A = g (x) rstd, B = b (x) 1 - g (x) mean*rstd).
"""

import sys

sys.path.insert(0, "/opt/trn_rl_repo")

import numpy as np
import concourse.tile as tile
import concourse.mybir as mybir
from concourse import bacc
from concourse.bass_utils import run_bass_kernel_spmd

f32 = mybir.dt.float32
f32r = mybir.dt.float32r
AF = mybir.ActivationFunctionType
OP = mybir.AluOpType

B, S, D, H, HD = 8, 1024, 1024, 16, 64
DF, O3, P = 2 * 1024, 3 * 1024, 128
EPS = 1e-5
KD = D // P           # 8 contraction chunks over D
KF = DF // P          # 16 contraction chunks over DF
SGW = 512             # matmul free-dim group width
NSG = S // SGW        # 2
NST = S // P          # 8
NCORES = 8
QSCALE = 1.0 / np.sqrt(HD)

_CACHE = {}


class _Pools:
    """Manual (stack-ordered) tile-pool lifetimes spanning phases."""

    def __init__(self, tc):
        self.tc = tc
        self._stack = []

    def open(self, name, bufs=1, space="SBUF"):
        cm = self.tc.tile_pool(name=name, bufs=bufs, space=space)
        pool = cm.__enter__()
        self._stack.append((name, cm))
        return pool

    def close(self, *names):
        """Close pools; must together form the top of the stack."""
        names = set(names)
        for _ in range(len(names)):
            top, cm = self._stack[-1]
            assert top in names, f"pool close order violation: {top} vs {names}"
            cm.__exit__(None, None, None)
            self._stack.pop()
            names.discard(top)


def _build_program():
    nc = bacc.Bacc("TRN2", target_bir_lowering=False, debug=False, num_devices=NCORES)

    def par(name, shape, dt, out=False):
        return nc.declare_dram_parameter(name, shape, dt, isOutput=out)

    xT = par("xT", [D, S], f32r)
    wqk = par("wqk", [16, P, KD, P], f32r)      # packed lhsT tiles for q,k
    wv = par("wv", [D, D], f32r)                # W_qkv.T[:, 2D:3D] natural [d, o]
    wout = par("wout", [KD, P, KD, P], f32r)
    w1 = par("w1", [KF, P, KD, P], f32r)
    w2 = par("w2", [KD, P, KF, P], f32r)
    bqkv_col = par("bqkv_col", [P, 16], f32)    # q,k bias per-partition (q pre-scaled)
    bqkv_row = par("bqkv_row", [1, D], f32r)    # v bias as row
    bout_col = par("bout_col", [P, KD], f32)
    b1_col = par("b1_col", [P, KF], f32)
    b2_col = par("b2_col", [P, KD], f32)
    ln1bg = par("ln1bg", [2, D], f32r)          # row0 = ln1_b, row1 = ln1_g
    ln1g_row = par("ln1g_row", [1, D], f32r)
    ln2bg = par("ln2bg", [2, D], f32r)
    ln2g_row = par("ln2g_row", [1, D], f32r)
    ones_s = par("ones_s", [1, S], f32r)
    eps_t = par("eps_t", [1, 1], f32)
    ones_col = par("ones_col", [P, 1], f32r)
    ones_row = par("ones_row", [1, P], f32r)
    vones = par("vones", [P, H], f32r)
    probsT = par("probsT", [H, S, S], f32, out=True)
    x2T = par("x2T", [D, S], f32, out=True)

    with tile.TileContext(nc) as tc:
        pl = _Pools(tc)
        cst = pl.open("cst", bufs=1)

        t_bqkv_col = cst.tile([P, 16], f32, tag="bqkvc", name="t_bqkv_col")
        nc.sync.dma_start(t_bqkv_col[:], bqkv_col[:])
        t_bqkv_row = cst.tile([1, D], f32r, tag="bqkvr", name="t_bqkv_row")
        nc.sync.dma_start(t_bqkv_row[:], bqkv_row[:])
        t_bout = cst.tile([P, KD], f32, tag="boutc", name="t_bout")
        nc.sync.dma_start(t_bout[:], bout_col[:])
        t_b1 = cst.tile([P, KF], f32, tag="b1c", name="t_b1")
        nc.sync.dma_start(t_b1[:], b1_col[:])
        t_b2 = cst.tile([P, KD], f32, tag="b2c", name="t_b2")
        nc.sync.dma_start(t_b2[:], b2_col[:])
        t_eps = cst.tile([1, 1], f32, tag="eps", name="t_eps")
        nc.sync.dma_start(t_eps[:], eps_t[:])
        t_ones_col = cst.tile([P, 1], f32r, tag="onesc", name="t_ones_col")
        nc.sync.dma_start(t_ones_col[:], ones_col[:])
        t_ones_row = cst.tile([1, P], f32r, tag="onesr", name="t_ones_row")
        nc.sync.dma_start(t_ones_row[:], ones_row[:])
        # ones_row copy living at partition 64: base-aligned with the sums row
        # of the ctx psum (see attention phase)
        t_ones_row64 = cst.tile([HD + 1, P], f32r, tag="onesr64", name="t_ones_row64")
        nc.sync.dma_start(t_ones_row64[HD:HD + 1, :], ones_row[:])

        # ================= Phase 2+3 scope: ctx =================
        ctxp = pl.open("ctxp", bufs=1)
        ctx_sb = [ctxp.tile([P, S], f32r, tag=f"c{k}", name=f"ctx{k}") for k in range(KD)]

        # ================= Phase 1+2 scope: qk / v =================
        qkp = pl.open("qkp", bufs=1)
        qk_sb = [qkp.tile([P, S], f32r, tag=f"qk{m}", name=f"qk{m}") for m in range(16)]
        vp = pl.open("vp", bufs=1)
        v_sb = vp.tile([P, NST, H, HD + 1], f32r, tag="v", name="v_sb")

        # ---------------- Phase 1: QKV ----------------
        xtp = pl.open("xtp", bufs=1)
        xT_sb = [xtp.tile([P, S], f32r, tag=f"x{k}", name=f"xt{k}") for k in range(KD)]
        for k in range(KD):
            nc.sync.dma_start(xT_sb[k][:], xT[k * P:(k + 1) * P, :])
        for st in range(NST):
            nc.gpsimd.dma_start(v_sb[:, st, :, HD:HD + 1], vones[:, :, None])

        wp = pl.open("wp", bufs=2)
        wvp = pl.open("wvp", bufs=1)
        ps1 = pl.open("ps1", bufs=4, space="PSUM")

        # v first (every attention head needs all of v): v[s, o] = x @ Wv + b
        OGW = 256  # narrow v o-groups keep the wv working set small
        for og in range(D // OGW):
            wv_t = [wvp.tile([P, OGW], f32r, tag=f"wv{k}", name=f"wv{k}") for k in range(KD)]
            for k in range(KD):
                nc.sync.dma_start(wv_t[k][:], wv[k * P:(k + 1) * P, og * OGW:(og + 1) * OGW])
            for st in range(NST):
                psum = ps1.tile([P, OGW], f32, tag="mmv", name="ps_v")
                nc.tensor.matmul(psum[:], t_ones_row[:],
                                 t_bqkv_row[0:1, og * OGW:(og + 1) * OGW],
                                 start=True, stop=False)
                for k in range(KD):
                    nc.tensor.matmul(psum[:], xT_sb[k][:, st * P:(st + 1) * P], wv_t[k][:],
                                     start=False, stop=(k == KD - 1))
                nc.vector.tensor_copy(
                    v_sb[:, st, og * 4:(og + 1) * 4, 0:HD],
                    psum[:].rearrange("p (h d) -> p h d", d=HD))

        # q,k in transposed layout: qkT[o, s] = Wqk @ x.T (+ bias, q pre-scaled)
        # paired order (q_j, k_j) so attention head-pairs unlock progressively
        for m in [j for i in range(8) for j in (i, 8 + i)]:
            wt = wp.tile([P, KD, P], f32r, tag="w", name="wqk_t")
            nc.sync.dma_start(wt[:], wqk[m])
            for sg in range(NSG):
                psum = ps1.tile([P, SGW], f32, tag="mm", name="ps_qk")
                for k in range(KD):
                    nc.tensor.matmul(psum[:], wt[:, k, :],
                                     xT_sb[k][:, sg * SGW:(sg + 1) * SGW],
                                     start=(k == 0), stop=(k == KD - 1))
                nc.scalar.activation(qk_sb[m][:, sg * SGW:(sg + 1) * SGW], psum[:],
                                     AF.Identity, bias=t_bqkv_col[:, m:m + 1],
                                     scale=QSCALE if m < 8 else 1.0)

        pl.close("ps1", "wvp", "wp", "xtp")

        # ---------------- Phase 2: attention ----------------
        expp = pl.open("expp", bufs=16)
        pnp = pl.open("pnp", bufs=8)
        bcp = pl.open("bcp", bufs=2)
        smp = pl.open("smp", bufs=2)
        psT = pl.open("psT", bufs=4, space="PSUM")
        psc = pl.open("psc", bufs=2, space="PSUM")

        for h in range(H):
            qt = qk_sb[h // 2]
            kt = qk_sb[8 + h // 2]
            hp = 64 * (h % 2)
            for sqh in range(NSG):
                sq = slice(sqh * SGW, (sqh + 1) * SGW)
                exps = []
                for skc in range(NST):
                    ps_ = psT.tile([P, SGW], f32, tag="sT", name="ps_sT")
                    nc.tensor.matmul(ps_[:], kt[hp:hp + 64, skc * P:(skc + 1) * P],
                                     qt[hp:hp + 64, sq], start=True, stop=True)
                    e = expp.tile([P, SGW], f32r, tag="exp", name="exp_t")
                    nc.scalar.activation(e[:], ps_[:], AF.Exp)
                    exps.append(e)
                pc = psc.tile([HD + 1, SGW], f32, tag="ctx", name="ps_ctx")
                for skc in range(NST):
                    nc.tensor.matmul(pc[:], v_sb[:, skc, h, :], exps[skc][:],
                                     start=(skc == 0), stop=(skc == NST - 1))
                # sums sit on psum partition 64 -> keep ops base-aligned
                rec_f = smp.tile([HD + 1, SGW], f32, tag="recf", name="rec_f")
                nc.vector.reciprocal(rec_f[HD:HD + 1, :], pc[HD:HD + 1, :])
                rec_r = smp.tile([HD + 1, SGW], f32r, tag="recr", name="rec_r")
                nc.gpsimd.tensor_copy(rec_r[HD:HD + 1, :], rec_f[HD:HD + 1, :])
                pb = psc.tile([P, SGW], f32, tag="bc", name="ps_bc")
                nc.tensor.matmul(pb[:], t_ones_row64[HD:HD + 1, :],
                                 rec_r[HD:HD + 1, :], start=True, stop=True)
                bcs = bcp.tile([P, SGW], f32, tag="bcs", name="bcs")
                nc.scalar.copy(bcs[:], pb[:])
                # normalized ctx^T into [d, s] tiles (head-pair halves)
                if h % 2 == 0:
                    nc.vector.tensor_tensor(ctx_sb[h // 2][0:64, sq], pc[0:64, :],
                                            bcs[0:64, :], OP.mult)
                else:
                    ctmp = smp.tile([64, SGW], f32r, tag="ctmp", name="ctmp")
                    nc.vector.tensor_tensor(ctmp[:], pc[0:64, :], bcs[0:64, :], OP.mult)
                    nc.sync.dma_start(ctx_sb[h // 2][64:128, sq], ctmp[:])
                # normalized probs -> DRAM (DVE reads the psum broadcast
                # directly; gpsimd has no PSUM port and reads the SBUF copy)
                for skc in range(NST):
                    pn = pnp.tile([P, SGW], f32, tag="pn", name="pn")
                    if skc < 5:
                        nc.vector.tensor_tensor(pn[:], exps[skc][:], pb[:], OP.mult)
                    else:
                        nc.gpsimd.tensor_tensor(pn[:], exps[skc][:], bcs[:], OP.mult)
                    nc.sync.dma_start(probsT[h, skc * P:(skc + 1) * P, sq], pn[:])

        pl.close("psc", "psT", "smp", "bcp", "pnp", "expp")
        pl.close("vp", "qkp")

        # ================= Phase 3..5 scope =================
        # x1 reuses the ctx tiles: every out-proj matmul (the only ctx reader)
        # completes before the LN1 apply (gated on full-resid1 stats) writes x1.
        x1_sb = ctx_sb

        def ln_block(resid, stats_pool, bc_mm_pool, lns, scratch_pool,
                     t_bg, t_g_row, writer):
            """LayerNorm over the partition (=feature) axis of KD [P,S] tiles.
            Emits out = resid * (g (x) rstd) + (b (x) 1 + g (x) (-mean*rstd))."""
            pstat = stats_pool.tile([1, S], f32, tag="st", name="pstat")
            pstat2 = stats_pool.tile([1, S], f32, tag="st", name="pstat2")
            for m in range(KD):
                for sg in range(NSG):
                    s_ = slice(sg * SGW, (sg + 1) * SGW)
                    sqv = scratch_pool.tile([P, SGW], f32r, tag="sq", name="sqv")
                    nc.scalar.activation(sqv[:], resid[m][:, s_], AF.Square)
                    nc.tensor.matmul(pstat[0:1, s_], t_ones_col[:], resid[m][:, s_],
                                     start=(m == 0), stop=(m == KD - 1))
                    nc.tensor.matmul(pstat2[0:1, s_], t_ones_col[:], sqv[:],
                                     start=(m == 0), stop=(m == KD - 1))
            mean_f = lns.tile([1, S], f32, tag="meanf", name="mean_f")
            nc.vector.tensor_scalar_mul(mean_f[:], pstat[:], 1.0 / D)
            ta = lns.tile([1, S], f32, tag="ta", name="ta")
            nc.vector.tensor_scalar_mul(ta[:], pstat2[:], 1.0 / D)
            tb = lns.tile([1, S], f32, tag="tb", name="tb")
            nc.vector.tensor_tensor(tb[:], mean_f[:], mean_f[:], OP.mult)
            nc.vector.tensor_tensor(ta[:], ta[:], tb[:], OP.subtract)      # var
            nc.scalar.activation(tb[:], ta[:], AF.Sqrt, bias=t_eps[0:1, 0:1])
            nc.vector.reciprocal(ta[:], tb[:])                             # rstd
            rstd_r = lns.tile([1, S], f32r, tag="rstdr", name="rstd_r")
            nc.gpsimd.tensor_copy(rstd_r[:], ta[:])
            # nmr = -mean * rstd (in-place over mean_f; row 1 of the B rhs)
            nc.vector.scalar_tensor_tensor(mean_f[:], mean_f[:], -1.0, ta[:],
                                           OP.mult, OP.mult)
            nmr_r = lns.tile([1, S], f32r, tag="nmrr", name="nmr_r")
            nc.gpsimd.tensor_copy(nmr_r[:], mean_f[:])
            bg_rhs = lns.tile([2, S], f32r, tag="bgrhs", name="bg_rhs")
            nc.sync.dma_start(bg_rhs[0:1, :], ones_s[:])
            nc.sync.dma_start(bg_rhs[1:2, :], nmr_r[:])
            for m in range(KD):
                for sg in range(NSG):
                    s_ = slice(sg * SGW, (sg + 1) * SGW)
                    pa = bc_mm_pool.tile([P, SGW], f32, tag="mm", name="ps_lnA")
                    nc.tensor.matmul(pa[:], t_g_row[0:1, m * P:(m + 1) * P],
                                     rstd_r[0:1, s_], start=True, stop=True)
                    pb_ = bc_mm_pool.tile([P, SGW], f32, tag="mm", name="ps_lnB")
                    nc.tensor.matmul(pb_[:], t_bg[:, m * P:(m + 1) * P],
                                     bg_rhs[:, s_], start=True, stop=True)
                    sc1 = scratch_pool.tile([P, SGW], f32, tag="scr", name="sc1")
                    if m % 2 == 0:
                        nc.vector.tensor_tensor(sc1[:], resid[m][:, s_], pa[:], OP.mult)
                        writer(m, sg, s_, sc1, pb_, nc.vector)
                    else:
                        # ACT evacuates the broadcasts so gpsimd (no PSUM
                        # port) can apply them off the critical DVE path
                        pa_s = scratch_pool.tile([P, SGW], f32, tag="pas", name="pa_s")
                        nc.scalar.copy(pa_s[:], pa[:])
                        pb_s = scratch_pool.tile([P, SGW], f32, tag="pbs", name="pb_s")
                        nc.scalar.copy(pb_s[:], pb_[:])
                        nc.gpsimd.tensor_tensor(sc1[:], resid[m][:, s_], pa_s[:], OP.mult)
                        writer(m, sg, s_, sc1, pb_s, nc.gpsimd)

        # ---------------- Phase 3: out-proj + residual + LN1 ----------------
        lnp3 = pl.open("lnp3", bufs=1)
        t_ln1bg = lnp3.tile([2, D], f32r, tag="lnbg", name="t_ln1bg")
        nc.sync.dma_start(t_ln1bg[:], ln1bg[:])
        t_ln1g_row = lnp3.tile([1, D], f32r, tag="lng", name="t_ln1g_row")
        nc.sync.dma_start(t_ln1g_row[:], ln1g_row[:])
        r1p = pl.open("r1p", bufs=1)
        resid1 = [r1p.tile([P, S], f32r, tag=f"r{k}", name=f"r1_{k}") for k in range(KD)]
        wop = pl.open("wop", bufs=2)
        xrp = pl.open("xrp", bufs=3)
        sqp = pl.open("sqp", bufs=3)
        lns3 = pl.open("lns3", bufs=1)
        mm3 = pl.open("mm3", bufs=4, space="PSUM")
        st3 = pl.open("st3", bufs=2, space="PSUM")

        for m in range(KD):
            wt = wop.tile([P, KD, P], f32r, tag="w", name="wout_t")
            nc.gpsimd.dma_start(wt[:], wout[m])
            for sg in range(NSG):
                s_ = slice(sg * SGW, (sg + 1) * SGW)
                xr = xrp.tile([P, SGW], f32r, tag="xr", name="xr")
                nc.gpsimd.dma_start(xr[:], xT[m * P:(m + 1) * P, s_])
                psum = mm3.tile([P, SGW], f32, tag="mm", name="ps_out")
                for k in range(KD):
                    nc.tensor.matmul(psum[:], wt[:, k, :], ctx_sb[k][:, s_],
                                     start=(k == 0), stop=(k == KD - 1))
                nc.vector.scalar_tensor_tensor(resid1[m][:, s_], psum[:],
                                               t_bout[:, m:m + 1], xr[:],
                                               OP.add, OP.add)

        def x1_writer(m, sg, s_, sc1, pb_, eng):
            eng.tensor_tensor(x1_sb[m][:, s_], sc1[:], pb_[:], OP.add)

        ln_block(resid1, st3, mm3, lns3, sqp, t_ln1bg, t_ln1g_row, x1_writer)

        pl.close("st3", "mm3", "lns3", "sqp", "xrp", "wop", "r1p", "lnp3")

        # ================= Phase 4+5 scope =================
        lnp5 = pl.open("lnp5", bufs=1)
        t_ln2bg = lnp5.tile([2, D], f32r, tag="lnbg", name="t_ln2bg")
        nc.sync.dma_start(t_ln2bg[:], ln2bg[:])
        t_ln2g_row = lnp5.tile([1, D], f32r, tag="lng", name="t_ln2g_row")
        nc.sync.dma_start(t_ln2g_row[:], ln2g_row[:])
        r2p = pl.open("r2p", bufs=1)
        resid2 = [r2p.tile([P, S], f32r, tag=f"r2{k}", name=f"r2_{k}") for k in range(KD)]
        sqp2 = pl.open("sqp2", bufs=2)
        lns5 = pl.open("lns5", bufs=1)
        x2o = pl.open("x2o", bufs=6)
        st5 = pl.open("st5", bufs=2, space="PSUM")

        hp_ = pl.open("hp", bufs=1)
        h_sb = [hp_.tile([P, S], f32r, tag=f"h{k}", name=f"h{k}") for k in range(KF)]

        # ---------------- Phase 4: FFN1 ----------------
        w1p = pl.open("w1p", bufs=3)
        mm4 = pl.open("mm4", bufs=4, space="PSUM")
        for m in range(KF):
            wt = w1p.tile([P, KD, P], f32r, tag="w", name="w1_t")
            nc.gpsimd.dma_start(wt[:], w1[m])
            for sg in range(NSG):
                s_ = slice(sg * SGW, (sg + 1) * SGW)
                psum = mm4.tile([P, SGW], f32, tag="mm", name="ps_ffn1")
                for k in range(KD):
                    nc.tensor.matmul(psum[:], wt[:, k, :], x1_sb[k][:, s_],
                                     start=(k == 0), stop=(k == KD - 1))
                nc.scalar.activation(h_sb[m][:, s_], psum[:], AF.Relu,
                                     bias=t_b1[:, m:m + 1])
        pl.close("mm4", "w1p")

        # ---------------- Phase 5: FFN2 + residual ----------------
        w2p = pl.open("w2p", bufs=3)
        mm5 = pl.open("mm5", bufs=4, space="PSUM")
        for m in range(KD):
            wta = w2p.tile([P, KD, P], f32r, tag="w", name="w2a_t")
            nc.sync.dma_start(wta[:], w2[m, :, 0:KD, :])
            wtb = w2p.tile([P, KD, P], f32r, tag="w", name="w2b_t")
            nc.sync.dma_start(wtb[:], w2[m, :, KD:KF, :])
            for sg in range(NSG):
                s_ = slice(sg * SGW, (sg + 1) * SGW)
                psum = mm5.tile([P, SGW], f32, tag="mm", name="ps_ffn2")
                for k in range(KF):
                    wt = wta if k < KD else wtb
                    nc.tensor.matmul(psum[:], wt[:, k % KD, :], h_sb[k][:, s_],
                                     start=(k == 0), stop=(k == KF - 1))
                nc.vector.scalar_tensor_tensor(resid2[m][:, s_], psum[:],
                                               t_b2[:, m:m + 1], x1_sb[m][:, s_],
                                               OP.add, OP.add)
        pl.close("mm5", "w2p", "hp")

        # ---------------- LN2 -> x2T ----------------
        bc5 = pl.open("bc5", bufs=4, space="PSUM")

        def x2_writer(m, sg, s_, sc1, pb_, eng):
            xo = x2o.tile([P, SGW], f32, tag="xo", name="xo")
            eng.tensor_tensor(xo[:], sc1[:], pb_[:], OP.add)
            nc.sync.dma_start(x2T[m * P:(m + 1) * P, s_], xo[:])

        ln_block(resid2, st5, bc5, lns5, sqp2, t_ln2bg, t_ln2g_row, x2_writer)

        pl.close("bc5")
        pl.close("st5", "x2o", "lns5", "sqp2", "r2p", "lnp5")
        pl.close("ctxp")
        pl.close("cst")

    nc.finalize()
    return nc


def _pack_lhsT(wt, kdim, mdim):
    """wt: [kdim, mdim] (contract-dim-major weight, i.e. W.T). Returns
    [mdim/128, 128, kdim/128, 128] with [m, p, ko, j] = wt[ko*128+p, m*128+j]."""
    return np.ascontiguousarray(
        wt.reshape(kdim // P, P, mdim // P, P).transpose(2, 1, 0, 3))


def _prep_shared(W_qkv, b_qkv, W_out, b_out, W1, b1, W2, b2,
                 ln1_g, ln1_b, ln2_g, ln2_b):
    WqkvT = np.ascontiguousarray(W_qkv.T.astype(np.float32))  # [D, 3D]
    shared = {
        "wqk": _pack_lhsT(WqkvT[:, :2 * D], D, 2 * D),
        "wv": np.ascontiguousarray(WqkvT[:, 2 * D:]),
        "wout": _pack_lhsT(np.ascontiguousarray(W_out.T.astype(np.float32)), D, D),
        "w1": _pack_lhsT(np.ascontiguousarray(W1.T.astype(np.float32)), D, DF),
        "w2": _pack_lhsT(np.ascontiguousarray(W2.T.astype(np.float32)), DF, D),
    }
    bqkv_col = b_qkv[:2 * D].reshape(16, P).T.astype(np.float32).copy()
    bqkv_col[:, :8] *= QSCALE
    shared["bqkv_col"] = bqkv_col
    shared["bqkv_row"] = b_qkv[2 * D:][None, :].astype(np.float32).copy()
    shared["bout_col"] = b_out.reshape(KD, P).T.astype(np.float32).copy()
    shared["b1_col"] = b1.reshape(KF, P).T.astype(np.float32).copy()
    shared["b2_col"] = b2.reshape(KD, P).T.astype(np.float32).copy()
    shared["ln1bg"] = np.stack([ln1_b, ln1_g]).astype(np.float32)
    shared["ln1g_row"] = ln1_g[None, :].astype(np.float32).copy()
    shared["ln2bg"] = np.stack([ln2_b, ln2_g]).astype(np.float32)
    shared["ln2g_row"] = ln2_g[None, :].astype(np.float32).copy()
    shared["ones_s"] = np.ones((1, S), np.float32)
    shared["eps_t"] = np.full((1, 1), EPS, np.float32)
    shared["ones_col"] = np.ones((P, 1), np.float32)
    shared["ones_row"] = np.ones((1, P), np.float32)
    shared["vones"] = np.ones((P, H), np.float32)
    return shared


def get_program():
    if "nc" not in _CACHE:
        _CACHE["nc"] = _build_program()
    return _CACHE["nc"]


def kernel(x, W_qkv, b_qkv, W_out, b_out, W1, b1, W2, b2,
           ln1_g, ln1_b, ln2_g, ln2_b, _return_raw=False):
    x = np.asarray(x, np.float32)
    nc = get_program()
    shared = _prep_shared(np.asarray(W_qkv), np.asarray(b_qkv), np.asarray(W_out),
                          np.asarray(b_out), np.asarray(W1), np.asarray(b1),
                          np.asarray(W2), np.asarray(b2), np.asarray(ln1_g),
                          np.asarray(ln1_b), np.asarray(ln2_g), np.asarray(ln2_b))
    in_maps = []
    for b in range(B):
        m = dict(shared)
        m["xT"] = np.ascontiguousarray(x[b].T)
        in_maps.append(m)
    res = run_bass_kernel_spmd(nc, in_maps, core_ids=list(range(NCORES)))
    x2 = np.stack([res.results[c]["x2T"].T for c in range(B)])
    probs = np.stack([res.results[c]["probsT"].transpose(0, 2, 1) for c in range(B)])
    if _return_raw:
        return x2, probs, res
    return x2, probs


# revision 30
# speedup vs baseline: 1.0082x; 1.0082x over previous
"""Transformer layer (attention with materialized probs + FFN) on 8 Trainium2
NeuronCores, data-parallel over the batch (B=8, one batch element per core).

Layout strategy: everything on-device is kept transposed ([feature, seq]) so
every matmul contracts over the SBUF partition dimension with weights in
natural (host-pre-transposed/packed) layout. Attention probs are produced as
probsT[h, sk, sq] and transposed back on the host during unsharding; x2 is
produced as x2T[d, s]. Matmuls run in float32r (TF32-like, ~1e-4 rel err,
full bf16-rate on the PE). Softmax skips the max-subtraction (scores are
O(1) by construction: q is pre-scaled by 1/sqrt(HD)); denominators come for
free from an all-ones 65th column appended to V in the probs@V matmul.
LayerNorm is applied as x*A + B with rank-1/rank-2 broadcast matmuls
(A = g (x) rstd, B = b (x) 1 - g (x) mean*rstd).
"""

import sys

sys.path.insert(0, "/opt/trn_rl_repo")

import numpy as np
import concourse.tile as tile
import concourse.mybir as mybir
from concourse import bacc
from concourse.bass_utils import run_bass_kernel_spmd

f32 = mybir.dt.float32
f32r = mybir.dt.float32r
AF = mybir.ActivationFunctionType
OP = mybir.AluOpType

B, S, D, H, HD = 8, 1024, 1024, 16, 64
DF, O3, P = 2 * 1024, 3 * 1024, 128
EPS = 1e-5
KD = D // P           # 8 contraction chunks over D
KF = DF // P          # 16 contraction chunks over DF
SGW = 512             # matmul free-dim group width
NSG = S // SGW        # 2
NST = S // P          # 8
NCORES = 8
QSCALE = 1.0 / np.sqrt(HD)

_CACHE = {}


class _Pools:
    """Manual (stack-ordered) tile-pool lifetimes spanning phases."""

    def __init__(self, tc):
        self.tc = tc
        self._stack = []

    def open(self, name, bufs=1, space="SBUF"):
        cm = self.tc.tile_pool(name=name, bufs=bufs, space=space)
        pool = cm.__enter__()
        self._stack.append((name, cm))
        return pool

    def close(self, *names):
        """Close pools; must together form the top of the stack."""
        names = set(names)
        for _ in range(len(names)):
            top, cm = self._stack[-1]
            assert top in names, f"pool close order violation: {top} vs {names}"
            cm.__exit__(None, None, None)
            self._stack.pop()
            names.discard(top)


def _build_program():
    nc = bacc.Bacc("TRN2", target_bir_lowering=False, debug=False, num_devices=NCORES)

    def par(name, shape, dt, out=False):
        return nc.declare_dram_parameter(name, shape, dt, isOutput=out)

    xT = par("xT", [D, S], f32r)
    wqk = par("wqk", [16, P, KD, P], f32r)      # packed lhsT tiles for q,k
    wv = par("wv", [D, D], f32r)                # W_qkv.T[:, 2D:3D] natural [d, o]
    wout = par("wout", [KD, P, KD, P], f32r)
    w1 = par("w1", [KF, P, KD, P], f32r)
    w2 = par("w2", [KD, P, KF, P], f32r)
    bqkv_col = par("bqkv_col", [P, 16], f32)    # q,k bias per-partition (q pre-scaled)
    bqkv_row = par("bqkv_row", [1, D], f32r)    # v bias as row
    bout_col = par("bout_col", [P, KD], f32)
    b1_col = par("b1_col", [P, KF], f32)
    b2_col = par("b2_col", [P, KD], f32)
    ln1bg = par("ln1bg", [2, D], f32r)          # row0 = ln1_b, row1 = ln1_g
    ln1g_row = par("ln1g_row", [1, D], f32r)
    ln2bg = par("ln2bg", [2, D], f32r)
    ln2g_row = par("ln2g_row", [1, D], f32r)
    ones_s = par("ones_s", [1, S], f32r)
    eps_t = par("eps_t", [1, 1], f32)
    ones_col = par("ones_col", [P, 1], f32r)
    ones_row = par("ones_row", [1, P], f32r)
    vones = par("vones", [P, H], f32r)
    probsT = par("probsT", [H, S, S], f32, out=True)
    x2T = par("x2T", [D, S], f32, out=True)

    with tile.TileContext(nc) as tc:
        pl = _Pools(tc)
        cst = pl.open("cst", bufs=1)

        t_bqkv_col = cst.tile([P, 16], f32, tag="bqkvc", name="t_bqkv_col")
        nc.sync.dma_start(t_bqkv_col[:], bqkv_col[:])
        t_bqkv_row = cst.tile([1, D], f32r, tag="bqkvr", name="t_bqkv_row")
        nc.sync.dma_start(t_bqkv_row[:], bqkv_row[:])
        t_bout = cst.tile([P, KD], f32, tag="boutc", name="t_bout")
        nc.sync.dma_start(t_bout[:], bout_col[:])
        t_b1 = cst.tile([P, KF], f32, tag="b1c", name="t_b1")
        nc.sync.dma_start(t_b1[:], b1_col[:])
        t_b2 = cst.tile([P, KD], f32, tag="b2c", name="t_b2")
        nc.sync.dma_start(t_b2[:], b2_col[:])
        t_eps = cst.tile([1, 1], f32, tag="eps", name="t_eps")
        nc.sync.dma_start(t_eps[:], eps_t[:])
        t_ones_col = cst.tile([P, 1], f32r, tag="onesc", name="t_ones_col")
        nc.sync.dma_start(t_ones_col[:], ones_col[:])
        t_ones_row = cst.tile([1, P], f32r, tag="onesr", name="t_ones_row")
        nc.sync.dma_start(t_ones_row[:], ones_row[:])
        # ones_row copy living at partition 64: base-aligned with the sums row
        # of the ctx psum (see attention phase)
        t_ones_row64 = cst.tile([HD + 1, P], f32r, tag="onesr64", name="t_ones_row64")
        nc.sync.dma_start(t_ones_row64[HD:HD + 1, :], ones_row[:])

        # ================= Phase 2+3 scope: ctx =================
        ctxp = pl.open("ctxp", bufs=1)
        ctx_sb = [ctxp.tile([P, S], f32r, tag=f"c{k}", name=f"ctx{k}") for k in range(KD)]

        # ================= Phase 1+2 scope: qk / v =================
        qkp = pl.open("qkp", bufs=1)
        qk_sb = [qkp.tile([P, S], f32r, tag=f"qk{m}", name=f"qk{m}") for m in range(16)]
        vp = pl.open("vp", bufs=1)
        v_sb = vp.tile([P, NST, H, HD + 1], f32r, tag="v", name="v_sb")

        # ---------------- Phase 1: QKV ----------------
        xtp = pl.open("xtp", bufs=1)
        xT_sb = [xtp.tile([P, S], f32r, tag=f"x{k}", name=f"xt{k}") for k in range(KD)]
        for k in range(KD):
            nc.sync.dma_start(xT_sb[k][:], xT[k * P:(k + 1) * P, :])
        for st in range(NST):
            nc.gpsimd.dma_start(v_sb[:, st, :, HD:HD + 1], vones[:, :, None])

        wp = pl.open("wp", bufs=3)
        wvp = pl.open("wvp", bufs=1)
        ps1 = pl.open("ps1", bufs=4, space="PSUM")

        # v first (every attention head needs all of v): v[s, o] = x @ Wv + b
        OGW = 256  # narrow v o-groups keep the wv working set small
        for og in range(D // OGW):
            wv_t = [wvp.tile([P, OGW], f32r, tag=f"wv{k}", name=f"wv{k}") for k in range(KD)]
            for k in range(KD):
                nc.sync.dma_start(wv_t[k][:], wv[k * P:(k + 1) * P, og * OGW:(og + 1) * OGW])
            for st in range(NST):
                psum = ps1.tile([P, OGW], f32, tag="mmv", name="ps_v")
                nc.tensor.matmul(psum[:], t_ones_row[:],
                                 t_bqkv_row[0:1, og * OGW:(og + 1) * OGW],
                                 start=True, stop=False)
                for k in range(KD):
                    nc.tensor.matmul(psum[:], xT_sb[k][:, st * P:(st + 1) * P], wv_t[k][:],
                                     start=False, stop=(k == KD - 1))
                nc.vector.tensor_copy(
                    v_sb[:, st, og * 4:(og + 1) * 4, 0:HD],
                    psum[:].rearrange("p (h d) -> p h d", d=HD))

        # q,k in transposed layout: qkT[o, s] = Wqk @ x.T (+ bias, q pre-scaled)
        # paired order (q_j, k_j) so attention head-pairs unlock progressively
        for m in [j for i in range(8) for j in (i, 8 + i)]:
            wt = wp.tile([P, KD, P], f32r, tag="w", name="wqk_t")
            nc.sync.dma_start(wt[:], wqk[m])
            for sg in range(NSG):
                psum = ps1.tile([P, SGW], f32, tag="mm", name="ps_qk")
                for k in range(KD):
                    nc.tensor.matmul(psum[:], wt[:, k, :],
                                     xT_sb[k][:, sg * SGW:(sg + 1) * SGW],
                                     start=(k == 0), stop=(k == KD - 1))
                nc.scalar.activation(qk_sb[m][:, sg * SGW:(sg + 1) * SGW], psum[:],
                                     AF.Identity, bias=t_bqkv_col[:, m:m + 1],
                                     scale=QSCALE if m < 8 else 1.0)

        pl.close("ps1", "wvp", "wp", "xtp")

        # ---------------- Phase 2: attention ----------------
        expp = pl.open("expp", bufs=16)
        pnp = pl.open("pnp", bufs=7)
        bcp = pl.open("bcp", bufs=2)
        smp = pl.open("smp", bufs=3)
        psT = pl.open("psT", bufs=4, space="PSUM")
        psc = pl.open("psc", bufs=2, space="PSUM")

        for h in range(H):
            qt = qk_sb[h // 2]
            kt = qk_sb[8 + h // 2]
            hp = 64 * (h % 2)
            for sqh in range(NSG):
                sq = slice(sqh * SGW, (sqh + 1) * SGW)
                exps = []
                for skc in range(NST):
                    ps_ = psT.tile([P, SGW], f32, tag="sT", name="ps_sT")
                    nc.tensor.matmul(ps_[:], kt[hp:hp + 64, skc * P:(skc + 1) * P],
                                     qt[hp:hp + 64, sq], start=True, stop=True)
                    e = expp.tile([P, SGW], f32r, tag="exp", name="exp_t")
                    nc.scalar.activation(e[:], ps_[:], AF.Exp)
                    exps.append(e)
                pc = psc.tile([HD + 1, SGW], f32, tag="ctx", name="ps_ctx")
                for skc in range(NST):
                    nc.tensor.matmul(pc[:], v_sb[:, skc, h, :], exps[skc][:],
                                     start=(skc == 0), stop=(skc == NST - 1))
                # sums sit on psum partition 64 -> keep ops base-aligned
                rec_f = smp.tile([HD + 1, SGW], f32, tag="recf", name="rec_f")
                nc.vector.reciprocal(rec_f[HD:HD + 1, :], pc[HD:HD + 1, :])
                rec_r = smp.tile([HD + 1, SGW], f32r, tag="recr", name="rec_r")
                nc.gpsimd.tensor_copy(rec_r[HD:HD + 1, :], rec_f[HD:HD + 1, :])
                pb = psc.tile([P, SGW], f32, tag="bc", name="ps_bc")
                nc.tensor.matmul(pb[:], t_ones_row64[HD:HD + 1, :],
                                 rec_r[HD:HD + 1, :], start=True, stop=True)
                bcs = bcp.tile([P, SGW], f32, tag="bcs", name="bcs")
                nc.scalar.copy(bcs[:], pb[:])
                # normalized ctx^T into [d, s] tiles (head-pair halves)
                if h % 2 == 0:
                    nc.vector.tensor_tensor(ctx_sb[h // 2][0:64, sq], pc[0:64, :],
                                            bcs[0:64, :], OP.mult)
                else:
                    ctmp = smp.tile([64, SGW], f32r, tag="ctmp", name="ctmp")
                    nc.vector.tensor_tensor(ctmp[:], pc[0:64, :], bcs[0:64, :], OP.mult)
                    nc.sync.dma_start(ctx_sb[h // 2][64:128, sq], ctmp[:])
                # normalized probs -> DRAM (DVE reads the psum broadcast
                # directly; gpsimd has no PSUM port and reads the SBUF copy)
                for skc in range(NST):
                    pn = pnp.tile([P, SGW], f32, tag="pn", name="pn")
                    if skc < 5:
                        nc.vector.tensor_tensor(pn[:], exps[skc][:], pb[:], OP.mult)
                    else:
                        nc.gpsimd.tensor_tensor(pn[:], exps[skc][:], bcs[:], OP.mult)
                    nc.sync.dma_start(probsT[h, skc * P:(skc + 1) * P, sq], pn[:])

        pl.close("psc", "psT", "smp", "bcp", "pnp", "expp")
        pl.close("vp", "qkp")

        # ================= Phase 3..5 scope =================
        # x1 reuses the ctx tiles: every out-proj matmul (the only ctx reader)
        # completes before the LN1 apply (gated on full-resid1 stats) writes x1.
        x1_sb = ctx_sb

        def ln_block(resid, stats_pool, bc_mm_pool, lns, scratch_pool,
                     t_bg, t_g_row, writer):
            """LayerNorm over the partition (=feature) axis of KD [P,S] tiles.
            Emits out = resid * (g (x) rstd) + (b (x) 1 + g (x) (-mean*rstd))."""
            pstat = stats_pool.tile([1, S], f32, tag="st", name="pstat")
            pstat2 = stats_pool.tile([1, S], f32, tag="st", name="pstat2")
            for m in range(KD):
                for sg in range(NSG):
                    s_ = slice(sg * SGW, (sg + 1) * SGW)
                    sqv = scratch_pool.tile([P, SGW], f32r, tag="sq", name="sqv")
                    nc.scalar.activation(sqv[:], resid[m][:, s_], AF.Square)
                    nc.tensor.matmul(pstat[0:1, s_], t_ones_col[:], resid[m][:, s_],
                                     start=(m == 0), stop=(m == KD - 1))
                    nc.tensor.matmul(pstat2[0:1, s_], t_ones_col[:], sqv[:],
                                     start=(m == 0), stop=(m == KD - 1))
            mean_f = lns.tile([1, S], f32, tag="meanf", name="mean_f")
            nc.vector.tensor_scalar_mul(mean_f[:], pstat[:], 1.0 / D)
            ta = lns.tile([1, S], f32, tag="ta", name="ta")
            nc.vector.tensor_scalar_mul(ta[:], pstat2[:], 1.0 / D)
            tb = lns.tile([1, S], f32, tag="tb", name="tb")
            nc.vector.tensor_tensor(tb[:], mean_f[:], mean_f[:], OP.mult)
            nc.vector.tensor_tensor(ta[:], ta[:], tb[:], OP.subtract)      # var
            nc.scalar.activation(tb[:], ta[:], AF.Sqrt, bias=t_eps[0:1, 0:1])
            nc.vector.reciprocal(ta[:], tb[:])                             # rstd
            rstd_r = lns.tile([1, S], f32r, tag="rstdr", name="rstd_r")
            nc.gpsimd.tensor_copy(rstd_r[:], ta[:])
            # nmr = -mean * rstd (in-place over mean_f; row 1 of the B rhs)
            nc.vector.scalar_tensor_tensor(mean_f[:], mean_f[:], -1.0, ta[:],
                                           OP.mult, OP.mult)
            nmr_r = lns.tile([1, S], f32r, tag="nmrr", name="nmr_r")
            nc.gpsimd.tensor_copy(nmr_r[:], mean_f[:])
            bg_rhs = lns.tile([2, S], f32r, tag="bgrhs", name="bg_rhs")
            nc.sync.dma_start(bg_rhs[0:1, :], ones_s[:])
            nc.sync.dma_start(bg_rhs[1:2, :], nmr_r[:])
            for m in range(KD):
                for sg in range(NSG):
                    s_ = slice(sg * SGW, (sg + 1) * SGW)
                    pa = bc_mm_pool.tile([P, SGW], f32, tag="mm", name="ps_lnA")
                    nc.tensor.matmul(pa[:], t_g_row[0:1, m * P:(m + 1) * P],
                                     rstd_r[0:1, s_], start=True, stop=True)
                    pb_ = bc_mm_pool.tile([P, SGW], f32, tag="mm", name="ps_lnB")
                    nc.tensor.matmul(pb_[:], t_bg[:, m * P:(m + 1) * P],
                                     bg_rhs[:, s_], start=True, stop=True)
                    sc1 = scratch_pool.tile([P, SGW], f32, tag="scr", name="sc1")
                    if m % 2 == 0:
                        nc.vector.tensor_tensor(sc1[:], resid[m][:, s_], pa[:], OP.mult)
                        writer(m, sg, s_, sc1, pb_, nc.vector)
                    else:
                        # ACT evacuates the broadcasts so gpsimd (no PSUM
                        # port) can apply them off the critical DVE path
                        pa_s = scratch_pool.tile([P, SGW], f32, tag="pas", name="pa_s")
                        nc.scalar.copy(pa_s[:], pa[:])
                        pb_s = scratch_pool.tile([P, SGW], f32, tag="pbs", name="pb_s")
                        nc.scalar.copy(pb_s[:], pb_[:])
                        nc.gpsimd.tensor_tensor(sc1[:], resid[m][:, s_], pa_s[:], OP.mult)
                        writer(m, sg, s_, sc1, pb_s, nc.gpsimd)

        # ---------------- Phase 3: out-proj + residual + LN1 ----------------
        lnp3 = pl.open("lnp3", bufs=1)
        t_ln1bg = lnp3.tile([2, D], f32r, tag="lnbg", name="t_ln1bg")
        nc.sync.dma_start(t_ln1bg[:], ln1bg[:])
        t_ln1g_row = lnp3.tile([1, D], f32r, tag="lng", name="t_ln1g_row")
        nc.sync.dma_start(t_ln1g_row[:], ln1g_row[:])
        r1p = pl.open("r1p", bufs=1)
        resid1 = [r1p.tile([P, S], f32r, tag=f"r{k}", name=f"r1_{k}") for k in range(KD)]
        wop = pl.open("wop", bufs=2)
        xrp = pl.open("xrp", bufs=3)
        sqp = pl.open("sqp", bufs=3)
        lns3 = pl.open("lns3", bufs=1)
        mm3 = pl.open("mm3", bufs=4, space="PSUM")
        st3 = pl.open("st3", bufs=2, space="PSUM")

        for m in range(KD):
            wt = wop.tile([P, KD, P], f32r, tag="w", name="wout_t")
            nc.gpsimd.dma_start(wt[:], wout[m])
            for sg in range(NSG):
                s_ = slice(sg * SGW, (sg + 1) * SGW)
                xr = xrp.tile([P, SGW], f32r, tag="xr", name="xr")
                nc.gpsimd.dma_start(xr[:], xT[m * P:(m + 1) * P, s_])
                psum = mm3.tile([P, SGW], f32, tag="mm", name="ps_out")
                for k in range(KD):
                    nc.tensor.matmul(psum[:], wt[:, k, :], ctx_sb[k][:, s_],
                                     start=(k == 0), stop=(k == KD - 1))
                nc.vector.scalar_tensor_tensor(resid1[m][:, s_], psum[:],
                                               t_bout[:, m:m + 1], xr[:],
                                               OP.add, OP.add)

        def x1_writer(m, sg, s_, sc1, pb_, eng):
            eng.tensor_tensor(x1_sb[m][:, s_], sc1[:], pb_[:], OP.add)

        ln_block(resid1, st3, mm3, lns3, sqp, t_ln1bg, t_ln1g_row, x1_writer)

        pl.close("st3", "mm3", "lns3", "sqp", "xrp", "wop", "r1p", "lnp3")

        # ================= Phase 4+5 scope =================
        lnp5 = pl.open("lnp5", bufs=1)
        t_ln2bg = lnp5.tile([2, D], f32r, tag="lnbg", name="t_ln2bg")
        nc.sync.dma_start(t_ln2bg[:], ln2bg[:])
        t_ln2g_row = lnp5.tile([1, D], f32r, tag="lng", name="t_ln2g_row")
        nc.sync.dma_start(t_ln2g_row[:], ln2g_row[:])
        r2p = pl.open("r2p", bufs=1)
        resid2 = [r2p.tile([P, S], f32r, tag=f"r2{k}", name=f"r2_{k}") for k in range(KD)]
        sqp2 = pl.open("sqp2", bufs=2)
        lns5 = pl.open("lns5", bufs=1)
        x2o = pl.open("x2o", bufs=6)
        st5 = pl.open("st5", bufs=2, space="PSUM")

        hp_ = pl.open("hp", bufs=1)
        h_sb = [hp_.tile([P, S], f32r, tag=f"h{k}", name=f"h{k}") for k in range(KF)]

        # ---------------- Phase 4: FFN1 ----------------
        w1p = pl.open("w1p", bufs=3)
        mm4 = pl.open("mm4", bufs=4, space="PSUM")
        for m in range(KF):
            wt = w1p.tile([P, KD, P], f32r, tag="w", name="w1_t")
            nc.gpsimd.dma_start(wt[:], w1[m])
            for sg in range(NSG):
                s_ = slice(sg * SGW, (sg + 1) * SGW)
                psum = mm4.tile([P, SGW], f32, tag="mm", name="ps_ffn1")
                for k in range(KD):
                    nc.tensor.matmul(psum[:], wt[:, k, :], x1_sb[k][:, s_],
                                     start=(k == 0), stop=(k == KD - 1))
                nc.scalar.activation(h_sb[m][:, s_], psum[:], AF.Relu,
                                     bias=t_b1[:, m:m + 1])
        pl.close("mm4", "w1p")

        # ---------------- Phase 5: FFN2 + residual ----------------
        w2p = pl.open("w2p", bufs=3)
        mm5 = pl.open("mm5", bufs=4, space="PSUM")
        for m in range(KD):
            wta = w2p.tile([P, KD, P], f32r, tag="w", name="w2a_t")
            nc.sync.dma_start(wta[:], w2[m, :, 0:KD, :])
            wtb = w2p.tile([P, KD, P], f32r, tag="w", name="w2b_t")
            nc.sync.dma_start(wtb[:], w2[m, :, KD:KF, :])
            for sg in range(NSG):
                s_ = slice(sg * SGW, (sg + 1) * SGW)
                psum = mm5.tile([P, SGW], f32, tag="mm", name="ps_ffn2")
                for k in range(KF):
                    wt = wta if k < KD else wtb
                    nc.tensor.matmul(psum[:], wt[:, k % KD, :], h_sb[k][:, s_],
                                     start=(k == 0), stop=(k == KF - 1))
                nc.vector.scalar_tensor_tensor(resid2[m][:, s_], psum[:],
                                               t_b2[:, m:m + 1], x1_sb[m][:, s_],
                                               OP.add, OP.add)
        pl.close("mm5", "w2p", "hp")

        # ---------------- LN2 -> x2T ----------------
        bc5 = pl.open("bc5", bufs=4, space="PSUM")

        def x2_writer(m, sg, s_, sc1, pb_, eng):
            xo = x2o.tile([P, SGW], f32, tag="xo", name="xo")
            eng.tensor_tensor(xo[:], sc1[:], pb_[:], OP.add)
            nc.sync.dma_start(x2T[m * P:(m + 1) * P, s_], xo[:])

        ln_block(resid2, st5, bc5, lns5, sqp2, t_ln2bg, t_ln2g_row, x2_writer)

        pl.close("bc5")
        pl.close("st5", "x2o", "lns5", "sqp2", "r2p", "lnp5")
        pl.close("ctxp")
        pl.close("cst")

    nc.finalize()
    return nc


def _pack_lhsT(wt, kdim, mdim):
    """wt: [kdim, mdim] (contract-dim-major weight, i.e. W.T). Returns
    [mdim/128, 128, kdim/128, 128] with [m, p, ko, j] = wt[ko*128+p, m*128+j]."""
    return np.ascontiguousarray(
        wt.reshape(kdim // P, P, mdim // P, P).transpose(2, 1, 0, 3))


def _prep_shared(W_qkv, b_qkv, W_out, b_out, W1, b1, W2, b2,
                 ln1_g, ln1_b, ln2_g, ln2_b):
    WqkvT = np.ascontiguousarray(W_qkv.T.astype(np.float32))  # [D, 3D]
    shared = {
        "wqk": _pack_lhsT(WqkvT[:, :2 * D], D, 2 * D),
        "wv": np.ascontiguousarray(WqkvT[:, 2 * D:]),
        "wout": _pack_lhsT(np.ascontiguousarray(W_out.T.astype(np.float32)), D, D),
        "w1": _pack_lhsT(np.ascontiguousarray(W1.T.astype(np.float32)), D, DF),
        "w2": _pack_lhsT(np.ascontiguousarray(W2.T.astype(np.float32)), DF, D),
    }
    bqkv_col = b_qkv[:2 * D].reshape(16, P).T.astype(np.float32).copy()
    bqkv_col[:, :8] *= QSCALE
    shared["bqkv_col"] = bqkv_col
    shared["bqkv_row"] = b_qkv[2 * D:][None, :].astype(np.float32).copy()
    shared["bout_col"] = b_out.reshape(KD, P).T.astype(np.float32).copy()
    shared["b1_col"] = b1.reshape(KF, P).T.astype(np.float32).copy()
    shared["b2_col"] = b2.reshape(KD, P).T.astype(np.float32).copy()
    shared["ln1bg"] = np.stack([ln1_b, ln1_g]).astype(np.float32)
    shared["ln1g_row"] = ln1_g[None, :].astype(np.float32).copy()
    shared["ln2bg"] = np.stack([ln2_b, ln2_g]).astype(np.float32)
    shared["ln2g_row"] = ln2_g[None, :].astype(np.float32).copy()
    shared["ones_s"] = np.ones((1, S), np.float32)
    shared["eps_t"] = np.full((1, 1), EPS, np.float32)
    shared["ones_col"] = np.ones((P, 1), np.float32)
    shared["ones_row"] = np.ones((1, P), np.float32)
    shared["vones"] = np.ones((P, H), np.float32)
    return shared


def get_program():
    if "nc" not in _CACHE:
        _CACHE["nc"] = _build_program()
    return _CACHE["nc"]


def kernel(x, W_qkv, b_qkv, W_out, b_out, W1, b1, W2, b2,
           ln1_g, ln1_b, ln2_g, ln2_b, _return_raw=False):
    x = np.asarray(x, np.float32)
    nc = get_program()
    shared = _prep_shared(np.asarray(W_qkv), np.asarray(b_qkv), np.asarray(W_out),
                          np.asarray(b_out), np.asarray(W1), np.asarray(b1),
                          np.asarray(W2), np.asarray(b2), np.asarray(ln1_g),
                          np.asarray(ln1_b), np.asarray(ln2_g), np.asarray(ln2_b))
    in_maps = []
    for b in range(B):
        m = dict(shared)
        m["xT"] = np.ascontiguousarray(x[b].T)
        in_maps.append(m)
    res = run_bass_kernel_spmd(nc, in_maps, core_ids=list(range(NCORES)))
    x2 = np.stack([res.results[c]["x2T"].T for c in range(B)])
    probs = np.stack([res.results[c]["probsT"].transpose(0, 2, 1) for c in range(B)])
    if _return_raw:
        return x2, probs, res
    return x2, probs
